# revision 1
# baseline (speedup 1.0000x reference)
"""Chamfer distance kernel for Trainium2 (Bass/Tile), SPMD over 8 NeuronCores.

Math (per batch b):
  dist[v,l] = ||x_v||^2 - 2 x_v.y_l + ||y_l||^2,  x=[1024,512], y=[512,512]
  out[b] = mean_v min_l dist + mean_l min_v dist

Strategy:
  - Data-parallel over batch: 64 batches -> 8 cores x 8 batches.
  - Host passes one tensor xy[b] = concat(-2*x^T, y^T) along the row dim
    ([D, Nv+Nl] per batch) so both matmul operands have the contraction
    dim D on partitions, loaded with one large DMA per batch.
  - Per batch on-chip (all matmuls in fp32r: fp32 storage, fast PE path):
      squares (one ACT pass) -> column-of-ones matmuls (PE) give
        a_row = 4*||x||^2 (rescaled 0.25 on the PSUM->SBUF copy) and
        b_row = ||y||^2 as [1, N] row vectors.
      main matmul (PE): pm[v-chunk] = sum_k (-2x)^T_k . y^T_k, plus one
        K=2 augmentation matmul adding a_v + b_l broadcast via
        stationary [ones; a] x moving [b; ones] => pm = full dist chunk.
      D1: free-dim min per chunk (DVE) -> [128,1] columns -> summed.
      D2: running elementwise min across chunks (DVE), then 4 PE
        transposes + free-dim mins to reduce across partitions.
  - Final: cross-partition sums via ones-matmul (exact fp32), scale,
    DMA out [1, 8] per core; host concatenates to [64].
"""

import numpy as np

N_CORES = 8
B = 8          # batches per core
D = 512        # feature dim
NV = 1024      # video clips
NL = 512       # language tokens
P = 128        # partitions
KC = D // P    # contraction chunks = 4
MC = NV // P   # v chunks = 8
NT = NV + NL   # combined x|y row length per k-chunk

IO_BUFS = 4

_CACHE = {}


def _build_bass():
    import concourse.bass as bass
    import concourse.mybir as mybir
    import concourse.tile as tile
    from concourse import bacc
    from concourse.masks import make_identity

    f32 = mybir.dt.float32
    f32r = mybir.dt.float32r
    ALU = mybir.AluOpType
    AX = mybir.AxisListType
    SQUARE = mybir.ActivationFunctionType.Square

    nc = bacc.Bacc(None)
    xy_h = nc.declare_dram_parameter("xy", [B, D, NT], f32r, isOutput=False)
    out_h = nc.declare_dram_parameter("out", [1, B], f32, isOutput=True)

    with tile.TileContext(nc) as tc:
        with (
            tc.tile_pool(name="const", bufs=1) as cpool,
            tc.tile_pool(name="io", bufs=IO_BUFS) as io,
            tc.tile_pool(name="work", bufs=2) as work,
            tc.tile_pool(name="acc", bufs=1) as accp,
            tc.tile_pool(name="ps", bufs=2, space="PSUM") as ps,
            tc.tile_pool(name="psn", bufs=1, space="PSUM") as psn,
        ):
            identity = cpool.tile([P, P], f32, tag="ident")
            make_identity(nc, identity)
            ones_f32 = cpool.tile([P, 1], f32, tag="onesf")
            nc.vector.memset(ones_f32, 1.0)
            # Memset can't write float32r (ISA check); produce f32r ones
            # via ACT copies, which round on write.
            ones_col = cpool.tile([P, 1], f32r, tag="ones")
            nc.scalar.copy(out=ones_col, in_=ones_f32)
            ones2_f32 = cpool.tile([2, NV], f32, tag="ones2f")
            nc.vector.memset(ones2_f32, 1.0)

            # Augmentation operands (double buffered by batch parity):
            #   aug_stat: partition 0 = ones, partition 1 = a_row
            #   aug_mov:  partition 0 = b_row, partition 1 = ones
            # K=2 contraction pairs 1*b_n + a_m*1. Engine writes must start
            # at a 32-aligned partition: ones rows come from the one-time
            # whole-tile copies, per-batch b lands on partition 0 via ACT,
            # per-batch a reaches partition 1 via a SBUF->SBUF DMA.
            aug_stat = [
                cpool.tile([2, NV], f32r, tag=f"augs{i}", name=f"aug_stat{i}")
                for i in range(2)
            ]
            aug_mov = [
                cpool.tile([2, NL], f32r, tag=f"augm{i}", name=f"aug_mov{i}")
                for i in range(2)
            ]
            for t in aug_stat:
                nc.scalar.copy(out=t, in_=ones2_f32)
            for t in aug_mov:
                nc.scalar.copy(out=t, in_=ones2_f32[:, :NL])

            d1sums = accp.tile([P, B], f32, tag="d1s")
            d2sums = accp.tile([P, B], f32, tag="d2s")
            dall = accp.tile([P, B], f32, tag="dall")
            out_sb = accp.tile([1, B], f32, tag="osb")

            def xsl(k, lo, hi):
                return slice(k * NT + lo, k * NT + hi)

            for b in range(B):
                xytile = io.tile([P, KC * NT], f32r, tag="xy")
                nc.sync.dma_start(
                    out=xytile[:, : 2 * NT],
                    in_=xy_h[b, : 2 * P].rearrange("(k p) n -> p k n", p=P),
                )
                nc.sync.dma_start(
                    out=xytile[:, 2 * NT :],
                    in_=xy_h[b, 2 * P :].rearrange("(k p) n -> p k n", p=P),
                )

                # Squared elements for the norms (two ACT passes, one
                # per DMA half), then pairwise k-chunk adds on the
                # otherwise-idle GPSIMD to halve the norm matmul count.
                sq = work.tile([P, KC * NT], f32r, tag="sq")
                nc.scalar.activation(
                    out=sq[:, : 2 * NT], in_=xytile[:, : 2 * NT], func=SQUARE
                )
                nc.scalar.activation(
                    out=sq[:, 2 * NT :], in_=xytile[:, 2 * NT :], func=SQUARE
                )
                sqh = work.tile([P, 2 * NT], f32r, tag="sqh")
                nc.gpsimd.tensor_tensor(
                    out=sqh[:, :NT], in0=sq[:, :NT], in1=sq[:, NT : 2 * NT],
                    op=ALU.add,
                )
                nc.gpsimd.tensor_tensor(
                    out=sqh[:, NT:], in0=sq[:, 2 * NT : 3 * NT],
                    in1=sq[:, 3 * NT :], op=ALU.add,
                )

                # Cross-partition (over d) sums via ones-column matmuls.
                a_ps = psn.tile([1, NV], f32, tag="aps")
                b_ps = psn.tile([1, NL], f32, tag="bps")
                for k in range(2):
                    for h in range(2):
                        nc.tensor.matmul(
                            out=a_ps[:, h * 512 : (h + 1) * 512],
                            lhsT=ones_col,
                            rhs=sqh[:, k * NT + h * 512 : k * NT + (h + 1) * 512],
                            start=(k == 0),
                            stop=(k == 1),
                        )
                    nc.tensor.matmul(
                        out=b_ps,
                        lhsT=ones_col,
                        rhs=sqh[:, k * NT + NV : (k + 1) * NT],
                        start=(k == 0),
                        stop=(k == 1),
                    )

                ast = aug_stat[b % 2]
                amv = aug_mov[b % 2]
                # xy x-part was pre-scaled by -2 on host, so sq sums give
                # 4*||x||^2; rescale by 0.25 on the PSUM->SBUF copy.
                a_sb = work.tile([1, NV], f32r, tag="asb")
                nc.scalar.mul(out=a_sb, in_=a_ps, mul=0.25)
                nc.sync.dma_start(out=ast[1:2, :], in_=a_sb)
                nc.scalar.copy(out=amv[0:1, :], in_=b_ps)

                Rt = work.tile([P, NL], f32, tag="R")
                d1c = work.tile([P, MC], f32, tag="d1c")
                d2c = work.tile([P, KC], f32, tag="d2c")

                for m in range(MC):
                    pm = ps.tile([P, NL], f32, tag="P", bufs=3)
                    for k in range(KC):
                        nc.tensor.matmul(
                            out=pm,
                            lhsT=xytile[:, xsl(k, m * P, (m + 1) * P)],
                            rhs=xytile[:, xsl(k, NV, NT)],
                            start=(k == 0),
                            stop=False,
                        )
                    nc.tensor.matmul(
                        out=pm,
                        lhsT=ast[:, m * P : (m + 1) * P],
                        rhs=amv,
                        start=False,
                        stop=True,
                    )
                    # D1: min over l (free dim) for the 128 v of this chunk.
                    nc.vector.tensor_reduce(
                        out=d1c[:, m : m + 1], in_=pm, axis=AX.X, op=ALU.min
                    )
                    # D2: running elementwise min across v-chunks.
                    if m == 0:
                        nc.vector.tensor_copy(out=Rt, in_=pm)
                    else:
                        nc.vector.tensor_tensor(out=Rt, in0=Rt, in1=pm, op=ALU.min)

                # D2: reduce across the remaining 128 partitions via PE
                # transposes then free-dim mins.
                t_ps = ps.tile([P, NL], f32, tag="T", bufs=1)
                for j in range(KC):
                    nc.tensor.transpose(
                        out=t_ps[:, j * P : (j + 1) * P],
                        in_=Rt[:, j * P : (j + 1) * P],
                        identity=identity,
                    )
                for j in range(KC):
                    nc.vector.tensor_reduce(
                        out=d2c[:, j : j + 1],
                        in_=t_ps[:, j * P : (j + 1) * P],
                        axis=AX.X,
                        op=ALU.min,
                    )

                nc.vector.tensor_reduce(
                    out=d1sums[:, b : b + 1], in_=d1c, axis=AX.X, op=ALU.add
                )
                nc.vector.tensor_reduce(
                    out=d2sums[:, b : b + 1], in_=d2c, axis=AX.X, op=ALU.add
                )

            # out[b] = (sum_p d1sums + 2 * sum_p d2sums) / 1024
            nc.vector.scalar_tensor_tensor(
                out=dall,
                in0=d2sums,
                scalar=2.0,
                in1=d1sums,
                op0=ALU.mult,
                op1=ALU.add,
            )
            f_ps = psn.tile([1, B], f32, tag="fin")
            nc.tensor.matmul(
                out=f_ps, lhsT=ones_f32, rhs=dall, start=True, stop=True
            )
            nc.scalar.mul(out=out_sb, in_=f_ps, mul=1.0 / NV)
            nc.sync.dma_start(out=out_h[:], in_=out_sb)

    # Bacc defers register allocation + wait-splitting to finalize();
    # the pjrt execution path expects an already-finalized module.
    nc.finalize()
    return nc


def _get_bass():
    if "nc" not in _CACHE:
        _CACHE["nc"] = _build_bass()
    return _CACHE["nc"]


def _run(in_maps, trace=False):
    from concourse.bass_utils import run_bass_kernel_spmd

    nc = _get_bass()
    return run_bass_kernel_spmd(nc, in_maps, list(range(N_CORES)), trace=trace)


def round_fp32r(x):
    """Round f32 to fp32r (sign + 8 exp + 11 mantissa bits, RNE) — the
    precision the PE uses for float32r operands."""
    u = x.view(np.uint32)
    low = u & np.uint32(0xFFF)
    base = u & ~np.uint32(0xFFF)
    odd = ((base >> np.uint32(12)) & np.uint32(1)).astype(bool)
    round_up = (low > 0x800) | ((low == 0x800) & odd)
    out = base + (round_up.astype(np.uint32) << np.uint32(12))
    return out.view(np.float32)


def make_in_maps(video_feat, lang_feat):
    video = np.asarray(video_feat, dtype=np.float32)
    lang = np.asarray(lang_feat, dtype=np.float32)
    assert video.shape == (N_CORES * B, NV, D), video.shape
    assert lang.shape == (N_CORES * B, NL, D), lang.shape
    in_maps = []
    for c in range(N_CORES):
        vb = video[c * B : (c + 1) * B]
        lb = lang[c * B : (c + 1) * B]
        xy = np.empty((B, D, NT), np.float32)
        np.multiply(np.transpose(vb, (0, 2, 1)), np.float32(-2.0), out=xy[:, :, :NV])
        xy[:, :, NV:] = np.transpose(lb, (0, 2, 1))
        in_maps.append({"xy": round_fp32r(xy)})
    return in_maps


def kernel(video_feat, lang_feat):
    res = _run(make_in_maps(video_feat, lang_feat), trace=False)
    outs = [res.results[c]["out"].reshape(-1) for c in range(N_CORES)]
    return np.concatenate(outs).astype(np.float32)



# revision 6
# speedup vs baseline: 1.2010x; 1.2010x over previous
"""Chamfer distance kernel for Trainium2 (Bass/Tile), SPMD over 8 NeuronCores.

Math (per batch b):
  dist[v,l] = ||x_v||^2 - 2 x_v.y_l + ||y_l||^2,  x=[1024,512], y=[512,512]
  out[b] = mean_v min_l dist + mean_l min_v dist

Strategy (fp8 DoubleRow pipeline):
  - Data-parallel over batch: 64 batches -> 8 cores x 8 batches.
  - Host packs xs = fp8_e4m3(-2x) and ys = fp8_e4m3(y) in k-chunk layout
    [P, KC, N] (contraction on partitions). Norms of the QUANTIZED points
    are carried as 4 extra fp8 contraction rows (one DoubleRow aug matmul
    per v-chunk) in hi/lo residual encoding:
      a = 64*fp8(a/64) + fp8(a - 64*fp8(a/64))   (error < +-0.5), same for b
    so PSUM receives the FULL dist matrix chunk.
  - PE per v-chunk: 2 fp8 DoubleRow matmuls (K=512) + 1 DoubleRow aug.
  - ACT: pure copies PSUM->SBUF fp16, two chunks (adjacent PSUM banks,
    one [P, 2, NL] tile) per instruction.
  - DVE: D1 = tensor_reduce(min) over the free dim per chunk (fp16);
    D2 = running elementwise fp16 min into a [P, 2, NL] accumulator pair
    (pair 0's ACT copy lands directly in the accumulator), merged to fp32.
  - D2 finale: 4 fp32 PE transposes + free-dim min reduces, then
    cross-partition sums via ones-matmul, scale, DMA [1, 8] out per core.
"""

import numpy as np

N_CORES = 8
B = 8          # batches per core
D = 512        # feature dim
NV = 1024      # video clips
NL = 512       # language tokens
P = 128        # partitions
KC = D // P    # contraction chunks = 4
MC = NV // P   # v chunks = 8

_CACHE = {}


def _build_bass():
    import concourse.bass as bass
    import concourse.mybir as mybir
    import concourse.tile as tile
    from concourse import bacc
    from concourse.masks import make_identity

    f32 = mybir.dt.float32
    f16 = mybir.dt.float16
    f8 = mybir.dt.float8e4
    ALU = mybir.AluOpType
    AX = mybir.AxisListType
    DR = mybir.MatmulPerfMode.DoubleRow

    nc = bacc.Bacc(None)
    xs_h = nc.declare_dram_parameter("xs", [B, P, KC, NV], f8, isOutput=False)
    ys_h = nc.declare_dram_parameter("ys", [B, P, KC, NL], f8, isOutput=False)
    as_h = nc.declare_dram_parameter("as_", [B, 2, 2, NV], f8, isOutput=False)
    am_h = nc.declare_dram_parameter("am", [B, 2, 2, NL], f8, isOutput=False)
    out_h = nc.declare_dram_parameter("out", [1, B], f32, isOutput=True)

    with tile.TileContext(nc) as tc:
        with (
            tc.tile_pool(name="const", bufs=1) as cpool,
            tc.tile_pool(name="io", bufs=3) as io,
            tc.tile_pool(name="work", bufs=3) as work,
            tc.tile_pool(name="acc", bufs=1) as accp,
            tc.tile_pool(name="ps", bufs=2, space="PSUM") as ps,
            tc.tile_pool(name="psn", bufs=2, space="PSUM") as psn,
        ):
            ident32 = cpool.tile([P, P], f32, tag="ident")
            make_identity(nc, ident32)
            ones_f32 = cpool.tile([P, 1], f32, tag="onesf")
            nc.vector.memset(ones_f32, 1.0)

            d1sums = accp.tile([P, B], f32, tag="d1s")
            d2sums = accp.tile([P, B], f32, tag="d2s")
            dall = accp.tile([P, B], f32, tag="dall")
            out_sb = accp.tile([1, B], f32, tag="osb")

            for b in range(B):
                xs_t = io.tile([P, KC, NV], f8, tag="xs")
                ys_t = io.tile([P, KC, NL], f8, tag="ys")
                as_t = io.tile([2, 2, NV], f8, tag="as")
                am_t = io.tile([2, 2, NL], f8, tag="am")
                nc.sync.dma_start(out=xs_t[:, :2], in_=xs_h[b, :, :2])
                nc.sync.dma_start(out=xs_t[:, 2:], in_=xs_h[b, :, 2:])
                nc.sync.dma_start(out=ys_t, in_=ys_h[b])
                nc.sync.dma_start(out=as_t, in_=as_h[b])
                nc.sync.dma_start(out=am_t, in_=am_h[b])

                d1c = work.tile([P, MC], f32, tag="d1c", bufs=2)
                d2c = work.tile([P, KC], f32, tag="d2c", bufs=2)
                # fp16 D2 accumulator pair: [:,0,:] even chunks, [:,1,:] odd.
                rt2 = work.tile([P, 2, NL], f16, tag="rt2", bufs=2)

                for pr in range(MC // 2):
                    pm2 = ps.tile([P, 2, NL], f32, tag="pm", bufs=2)
                    for j in range(2):
                        m = 2 * pr + j
                        pm = pm2[:, j, :]
                        for kt2 in range(2):
                            nc.tensor.matmul(
                                out=pm,
                                lhsT=xs_t[:, 2 * kt2 : 2 * kt2 + 2, m * P : (m + 1) * P],
                                rhs=ys_t[:, 2 * kt2 : 2 * kt2 + 2, :],
                                start=(kt2 == 0),
                                stop=False,
                                perf_mode=DR,
                            )
                        # norm aug: adds a_v + b_l (hi/lo fp8 encoding).
                        nc.tensor.matmul(
                            out=pm,
                            lhsT=as_t[:, :, m * P : (m + 1) * P],
                            rhs=am_t,
                            start=False,
                            stop=True,
                            perf_mode=DR,
                        )
                    # PSUM -> SBUF fp16, both chunks in one ACT op.
                    c2 = rt2 if pr == 0 else work.tile([P, 2, NL], f16, tag="c2", bufs=3)
                    nc.scalar.copy(out=c2, in_=pm2)
                    # D1: free-dim min per chunk.
                    for j in range(2):
                        m = 2 * pr + j
                        nc.vector.tensor_reduce(
                            out=d1c[:, m : m + 1],
                            in_=c2[:, j, :],
                            axis=AX.X,
                            op=ALU.min,
                        )
                    # D2: running min into the accumulator pair.
                    if pr > 0:
                        for j in range(2):
                            nc.vector.tensor_tensor(
                                out=rt2[:, j, :],
                                in0=c2[:, j, :],
                                in1=rt2[:, j, :],
                                op=ALU.min,
                            )

                # Merge accumulator pair, converting to fp32 on the write.
                rtf = work.tile([P, NL], f32, tag="rtf", bufs=2)
                nc.vector.tensor_tensor(
                    out=rtf, in0=rt2[:, 0, :], in1=rt2[:, 1, :], op=ALU.min
                )
                t_ps = psn.tile([P, NL], f32, tag="t", bufs=2)
                for j in range(KC):
                    nc.tensor.transpose(
                        out=t_ps[:, j * P : (j + 1) * P],
                        in_=rtf[:, j * P : (j + 1) * P],
                        identity=ident32,
                    )
                for j in range(KC):
                    nc.vector.tensor_reduce(
                        out=d2c[:, j : j + 1],
                        in_=t_ps[:, j * P : (j + 1) * P],
                        axis=AX.X,
                        op=ALU.min,
                    )
                nc.vector.tensor_reduce(
                    out=d1sums[:, b : b + 1], in_=d1c, axis=AX.X, op=ALU.add
                )
                nc.vector.tensor_reduce(
                    out=d2sums[:, b : b + 1], in_=d2c, axis=AX.X, op=ALU.add
                )

            # out[b] = (sum_p d1sums + 2 * sum_p d2sums) / 1024
            nc.vector.scalar_tensor_tensor(
                out=dall,
                in0=d2sums,
                scalar=2.0,
                in1=d1sums,
                op0=ALU.mult,
                op1=ALU.add,
            )
            f_ps = psn.tile([1, B], f32, tag="fin", bufs=1)
            nc.tensor.matmul(
                out=f_ps, lhsT=ones_f32, rhs=dall, start=True, stop=True
            )
            nc.scalar.mul(out=out_sb, in_=f_ps, mul=1.0 / NV)
            nc.sync.dma_start(out=out_h[:], in_=out_sb)

    nc.finalize()
    return nc


def _get_bass():
    if "nc" not in _CACHE:
        _CACHE["nc"] = _build_bass()
    return _CACHE["nc"]


def _run(in_maps, trace=False):
    from concourse.bass_utils import run_bass_kernel_spmd

    nc = _get_bass()
    return run_bass_kernel_spmd(nc, in_maps, list(range(N_CORES)), trace=trace)


def make_in_maps(video_feat, lang_feat):
    import ml_dtypes

    f8 = ml_dtypes.float8_e4m3
    video = np.asarray(video_feat, dtype=np.float32)
    lang = np.asarray(lang_feat, dtype=np.float32)
    assert video.shape == (N_CORES * B, NV, D), video.shape
    assert lang.shape == (N_CORES * B, NL, D), lang.shape
    NB = N_CORES * B

    # Quantize once for all batches.
    xs8 = (-2.0 * video).astype(f8)                      # [64, NV, D]
    ys8 = lang.astype(f8)                                # [64, NL, D]
    xsf = xs8.astype(np.float32)
    ysf = ys8.astype(np.float32)
    a = np.einsum("bvd,bvd->bv", xsf, xsf) / 4.0         # ||x_q||^2  [64, NV]
    bn = np.einsum("bld,bld->bl", ysf, ysf)              # ||y_q||^2  [64, NL]

    def hi_lo(v):
        hi = (v / 64.0).astype(f8)
        lo = (v - 64.0 * hi.astype(np.float32)).astype(f8)
        return hi, lo

    a_hi, a_lo = hi_lo(a)
    b_hi, b_lo = hi_lo(bn)

    # aug stationary [64, 2, 2, NV]: tile0 = [64s; a_hi], tile1 = [1s; a_lo]
    as_dev = np.empty((NB, 2, 2, NV), f8)
    as_dev[:, 0, 0, :] = np.float32(64.0)
    as_dev[:, 1, 0, :] = a_hi
    as_dev[:, 0, 1, :] = np.float32(1.0)
    as_dev[:, 1, 1, :] = a_lo
    # aug moving [64, 2, 2, NL]: tile0 = [b_hi; 64s], tile1 = [b_lo; 1s]
    am_dev = np.empty((NB, 2, 2, NL), f8)
    am_dev[:, 0, 0, :] = b_hi
    am_dev[:, 1, 0, :] = np.float32(64.0)
    am_dev[:, 0, 1, :] = b_lo
    am_dev[:, 1, 1, :] = np.float32(1.0)

    # Device layouts: [P, KC, N] with element (p, kt, n) = op[n, kt*P+p].
    xs_dev = np.ascontiguousarray(
        xs8.reshape(NB, NV, KC, P).transpose(0, 3, 2, 1)
    )  # [64, P, KC, NV]
    ys_dev = np.ascontiguousarray(
        ys8.reshape(NB, NL, KC, P).transpose(0, 3, 2, 1)
    )  # [64, P, KC, NL]

    in_maps = []
    for c in range(N_CORES):
        sl = slice(c * B, (c + 1) * B)
        in_maps.append(
            {
                "xs": xs_dev[sl],
                "ys": ys_dev[sl],
                "as_": as_dev[sl],
                "am": am_dev[sl],
            }
        )
    return in_maps


def kernel(video_feat, lang_feat):
    in_maps = make_in_maps(video_feat, lang_feat)
    res = _run(in_maps, trace=False)
    outs = [res.results[c]["out"].reshape(-1) for c in range(N_CORES)]
    return np.concatenate(outs).astype(np.float32)


# revision 8
# speedup vs baseline: 1.7444x; 1.4525x over previous
"""Chamfer distance kernel for Trainium2 (Bass/Tile), SPMD over 8 NeuronCores.

Math (per batch b):
  dist[v,l] = ||x_v||^2 - 2 x_v.y_l + ||y_l||^2,  x=[1024,512], y=[512,512]
  out[b] = mean_v min_l dist + mean_l min_v dist

Strategy (fp8 DoubleRow pipeline, v5):
  - Data-parallel over batch: 64 batches -> 8 cores x 8 batches.
  - Host packs xs = fp8_e4m3(-2x) and ys = fp8_e4m3(y) in k-chunk layout
    [P, KC, N] (contraction on partitions). Norms of the QUANTIZED points
    ride 4 extra fp8 contraction rows in hi/lo residual encoding
      a = 64*fp8(a/64) + fp8(a - 64*fp8(a/64))   (error < +-0.5), same b
    inside a K=128 zero-padded plain-fp8 aug matmul per v-chunk (small-K
    matmuls cost ~1.6x a K=128 one on HW, so padding is a win).
  - PE per v-chunk: 2 fp8 DoubleRow matmuls (K=512, ~2 rows/cycle at
    steady state) + 1 padded aug -> PSUM holds the full dist chunk.
  - ACT: pure paired copies PSUM->SBUF fp16 (two chunks per op).
  - DVE (few, big ops): D1 = free-dim min reduce over [P,2,NL] (pair 0)
    and [P,6,NL] (rest); D2 = running elementwise fp16 min into a
    [P,2,NL] accumulator pair; merge converts to fp32.
  - D2 finale per batch (software-pipelined one batch late so the PE
    never stalls on it): 4 fp32 PE transposes + one [P,4,P] min reduce.
  - End: cross-batch reduce of d1/d2 accumulators, ones-matmul
    cross-partition sum, scale, DMA [1, 8] per core.
"""

import numpy as np

N_CORES = 8
B = 8          # batches per core
D = 512        # feature dim
NV = 1024      # video clips
NL = 512       # language tokens
P = 128        # partitions
KC = D // P    # contraction chunks = 4
MC = NV // P   # v chunks = 8

_CACHE = {}


def _build_bass():
    import concourse.bass as bass
    import concourse.mybir as mybir
    import concourse.tile as tile
    from concourse import bacc
    from concourse.masks import make_identity

    f32 = mybir.dt.float32
    f16 = mybir.dt.float16
    f8 = mybir.dt.float8e4
    ALU = mybir.AluOpType
    AX = mybir.AxisListType
    DR = mybir.MatmulPerfMode.DoubleRow

    nc = bacc.Bacc(None)
    xs_h = nc.declare_dram_parameter("xs", [B, P, KC, NV], f8, isOutput=False)
    ys_h = nc.declare_dram_parameter("ys", [B, P, KC, NL], f8, isOutput=False)
    as_h = nc.declare_dram_parameter("as_", [B, P, NV], f8, isOutput=False)
    am_h = nc.declare_dram_parameter("am", [B, P, NL], f8, isOutput=False)
    out_h = nc.declare_dram_parameter("out", [1, B], f32, isOutput=True)

    with tile.TileContext(nc) as tc:
        with (
            tc.tile_pool(name="const", bufs=1) as cpool,
            tc.tile_pool(name="io", bufs=3) as io,
            tc.tile_pool(name="work", bufs=2) as work,
            tc.tile_pool(name="acc", bufs=1) as accp,
            tc.tile_pool(name="ps", bufs=3, space="PSUM") as ps,
            tc.tile_pool(name="psn", bufs=1, space="PSUM") as psn,
        ):
            ident32 = cpool.tile([P, P], f32, tag="ident")
            make_identity(nc, ident32)
            ones_f32 = cpool.tile([P, 1], f32, tag="onesf")
            nc.vector.memset(ones_f32, 1.0)

            d1call = accp.tile([P, B, MC], f32, tag="d1call")
            d2call = accp.tile([P, B, KC], f32, tag="d2call")
            d1sums = accp.tile([P, B], f32, tag="d1s")
            d2sums = accp.tile([P, B], f32, tag="d2s")
            dall = accp.tile([P, B], f32, tag="dall")
            out_sb = accp.tile([1, B], f32, tag="osb")

            finale_args = [None] * B

            def issue_finale(bi):
                rt2p, rtfp = finale_args[bi]
                # Merge accumulator pair -> fp32 for the PE transpose.
                nc.vector.tensor_tensor(
                    out=rtfp, in0=rt2p[:, 0, :], in1=rt2p[:, 1, :], op=ALU.min
                )
                t_ps = psn.tile([P, KC, P], f32, tag="t", bufs=1)
                for j in range(KC):
                    nc.tensor.transpose(
                        out=t_ps[:, j, :],
                        in_=rtfp[:, j * P : (j + 1) * P],
                        identity=ident32,
                    )
                nc.vector.tensor_reduce(
                    out=d2call[:, bi, :], in_=t_ps, axis=AX.X, op=ALU.min
                )

            for b in range(B):
                xs_t = io.tile([P, KC, NV], f8, tag="xs")
                ys_t = io.tile([P, KC, NL], f8, tag="ys")
                as_t = io.tile([P, NV], f8, tag="as")
                am_t = io.tile([P, NL], f8, tag="am")
                nc.sync.dma_start(out=xs_t[:, :2], in_=xs_h[b, :, :2])
                nc.sync.dma_start(out=xs_t[:, 2:], in_=xs_h[b, :, 2:])
                nc.sync.dma_start(out=ys_t, in_=ys_h[b])
                nc.sync.dma_start(out=as_t, in_=as_h[b])
                nc.sync.dma_start(out=am_t, in_=am_h[b])

                c_all = work.tile([P, MC, NL], f16, tag="call", bufs=2)
                rt2 = work.tile([P, 2, NL], f16, tag="rt2", bufs=2)
                rtf = work.tile([P, NL], f32, tag="rtf", bufs=2)
                finale_args[b] = (rt2, rtf)

                for pr in range(MC // 2):
                    pm2 = ps.tile([P, 2, NL], f32, tag="pm", bufs=3)
                    for j in range(2):
                        m = 2 * pr + j
                        pm = pm2[:, j, :]
                        for kt2 in range(2):
                            nc.tensor.matmul(
                                out=pm,
                                lhsT=xs_t[:, 2 * kt2 : 2 * kt2 + 2, m * P : (m + 1) * P],
                                rhs=ys_t[:, 2 * kt2 : 2 * kt2 + 2, :],
                                start=(kt2 == 0),
                                stop=False,
                                perf_mode=DR,
                            )
                        # norm aug (a_v + b_l), K=128 zero-padded plain fp8.
                        nc.tensor.matmul(
                            out=pm,
                            lhsT=as_t[:, m * P : (m + 1) * P],
                            rhs=am_t,
                            start=False,
                            stop=True,
                        )
                    if pr == 0:
                        # Pair 0 lands directly in the D2 accumulator; D1 for
                        # chunks 0-1 reads it before any D2 min overwrites.
                        nc.scalar.copy(out=rt2, in_=pm2)
                        nc.vector.tensor_reduce(
                            out=d1call[:, b, 0:2], in_=rt2, axis=AX.X, op=ALU.min
                        )
                    else:
                        cpair = c_all[:, 2 * pr : 2 * pr + 2, :]
                        nc.scalar.copy(out=cpair, in_=pm2)
                        nc.vector.tensor_tensor(
                            out=rt2, in0=cpair, in1=rt2, op=ALU.min
                        )
                    if pr == 1 and b > 0:
                        # Software-pipelined finale of the previous batch:
                        # its DVE merge is ready by now, so the PE transposes
                        # slot between this batch's matmul pairs stall-free.
                        issue_finale(b - 1)

                nc.vector.tensor_reduce(
                    out=d1call[:, b, 2:MC],
                    in_=c_all[:, 2:MC, :],
                    axis=AX.X,
                    op=ALU.min,
                )

            issue_finale(B - 1)

            # Cross-batch: d1sums/d2sums [P, B] <- min-chunk sums.
            nc.vector.tensor_reduce(
                out=d1sums, in_=d1call, axis=AX.X, op=ALU.add
            )
            nc.vector.tensor_reduce(
                out=d2sums, in_=d2call, axis=AX.X, op=ALU.add
            )
            # out[b] = (sum_p d1sums + 2 * sum_p d2sums) / 1024
            nc.vector.scalar_tensor_tensor(
                out=dall,
                in0=d2sums,
                scalar=2.0,
                in1=d1sums,
                op0=ALU.mult,
                op1=ALU.add,
            )
            f_ps = psn.tile([1, B], f32, tag="fin", bufs=1)
            nc.tensor.matmul(
                out=f_ps, lhsT=ones_f32, rhs=dall, start=True, stop=True
            )
            nc.scalar.mul(out=out_sb, in_=f_ps, mul=1.0 / NV)
            nc.sync.dma_start(out=out_h[:], in_=out_sb)

    nc.finalize()
    return nc


def _get_bass():
    if "nc" not in _CACHE:
        _CACHE["nc"] = _build_bass()
    return _CACHE["nc"]


def _run(in_maps, trace=False):
    from concourse.bass_utils import run_bass_kernel_spmd

    nc = _get_bass()
    return run_bass_kernel_spmd(nc, in_maps, list(range(N_CORES)), trace=trace)


def make_in_maps(video_feat, lang_feat):
    import ml_dtypes

    f8 = ml_dtypes.float8_e4m3
    video = np.asarray(video_feat, dtype=np.float32)
    lang = np.asarray(lang_feat, dtype=np.float32)
    assert video.shape == (N_CORES * B, NV, D), video.shape
    assert lang.shape == (N_CORES * B, NL, D), lang.shape
    NB = N_CORES * B

    # Quantize once for all batches.
    xs8 = (-2.0 * video).astype(f8)                      # [64, NV, D]
    ys8 = lang.astype(f8)                                # [64, NL, D]
    xsf = xs8.astype(np.float32)
    ysf = ys8.astype(np.float32)
    a = np.einsum("bvd,bvd->bv", xsf, xsf) / 4.0         # ||x_q||^2  [64, NV]
    bn = np.einsum("bld,bld->bl", ysf, ysf)              # ||y_q||^2  [64, NL]

    def hi_lo(v):
        hi = (v / 64.0).astype(f8)
        lo = (v - 64.0 * hi.astype(np.float32)).astype(f8)
        return hi, lo

    a_hi, a_lo = hi_lo(a)
    b_hi, b_lo = hi_lo(bn)

    # aug stationary [64, P, NV]: rows (64s, a_hi, 1s, a_lo), rest zero.
    as_dev = np.zeros((NB, P, NV), f8)
    as_dev[:, 0, :] = np.float32(64.0)
    as_dev[:, 1, :] = a_hi
    as_dev[:, 2, :] = np.float32(1.0)
    as_dev[:, 3, :] = a_lo
    # aug moving [64, P, NL]: rows (b_hi, 64s, b_lo, 1s), rest zero.
    am_dev = np.zeros((NB, P, NL), f8)
    am_dev[:, 0, :] = b_hi
    am_dev[:, 1, :] = np.float32(64.0)
    am_dev[:, 2, :] = b_lo
    am_dev[:, 3, :] = np.float32(1.0)

    # Device layouts: [P, KC, N] with element (p, kt, n) = op[n, kt*P+p].
    xs_dev = np.ascontiguousarray(
        xs8.reshape(NB, NV, KC, P).transpose(0, 3, 2, 1)
    )  # [64, P, KC, NV]
    ys_dev = np.ascontiguousarray(
        ys8.reshape(NB, NL, KC, P).transpose(0, 3, 2, 1)
    )  # [64, P, KC, NL]

    in_maps = []
    for c in range(N_CORES):
        sl = slice(c * B, (c + 1) * B)
        in_maps.append(
            {
                "xs": xs_dev[sl],
                "ys": ys_dev[sl],
                "as_": as_dev[sl],
                "am": am_dev[sl],
            }
        )
    return in_maps


def kernel(video_feat, lang_feat):
    in_maps = make_in_maps(video_feat, lang_feat)
    res = _run(in_maps, trace=False)
    outs = [res.results[c]["out"].reshape(-1) for c in range(N_CORES)]
    return np.concatenate(outs).astype(np.float32)
